# revision 22
# baseline (speedup 1.0000x reference)
"""Trainium2 Bass kernel for the multimodal LSTHM recurrent net.

Strategy: pure data-parallel over 8 cores (batch 2048 -> 256/core).
Activations are kept feature-major [feat<=128 partitions, batch free] so
every matmul is out[feat_out, b] = W[feat_in, feat_out].T @ act[feat_in, b].
Matmul inputs are bf16 (fp32 PSUM accumulation, fp32 cell state c), which
measured ~5e-3 scale-relative output error in a numpy bit-accurate sim.
"""

import os
import sys

import numpy as np
import ml_dtypes

sys.path.insert(0, "/opt/trn_rl_repo")

bf16 = ml_dtypes.bfloat16

B, T = 2048, 100
NCORES = 8
BS = B // NCORES           # 256 batch per core
DL, DA, DV = 300, 74, 47
DH = 128
ZD = 128
K = 4
ZF1, ZF2 = 256, 256
FD = 128

XVA = DV + DA + 1          # stacked x_v | x_a | ones row = 122
XL_AUG = DL + 1            # x_l plus ones feature = 301

# gate order in the packed PSUM tensor: 9 sigmoid slices then 3 tanh slices
# slices: [i_v i_a i_l f_v f_a f_l o_v o_a o_l | g_v g_a g_l]
# reference weight column order is [i, f, g, o] blocks of 128
_QCOL = {"i": 0, "f": 1, "g": 2, "o": 3}
_SIG_Q = ["i", "f", "o"]


# ---------------------------------------------------------------- weights ---

def _pack_weights(params):
    """Pack every stationary operand into one [128, NW] bf16 matrix.

    Returns (w_np, offsets) where offsets[name] = (col, K, M).
    """
    p = {k: (v if isinstance(v, dict) else np.asarray(v, np.float32))
         for k, v in params.items()}
    for k, v in list(p.items()):
        if isinstance(v, dict):
            p[k] = {kk: np.asarray(vv, np.float32) for kk, vv in v.items()}

    tiles = []  # (name, arr[K, M])

    def add(name, arr):
        arr = np.asarray(arr, np.float32)
        assert arr.ndim == 2 and arr.shape[0] <= 128 and arr.shape[1] <= 128, (
            name, arr.shape)
        tiles.append((name, arr))

    # xva gate precompute: block diag with bias row and a self-ones column
    wg = np.zeros((XVA, XVA), np.float32)
    wg[0:DV, 0:DV] = p["gate_v_W"]
    wg[DV:DV + DA, DV:DV + DA] = p["gate_a_W"]
    wg[XVA - 1, 0:DV] = p["gate_v_b"]
    wg[XVA - 1, DV:DV + DA] = p["gate_a_b"]
    wg[XVA - 1, XVA - 1] = 30.0        # sigmoid(30) == 1 -> ones row survives
    add("wg", wg)

    # LSTHM gate weights
    for q in ["i", "f", "o", "g"]:
        qc = slice(_QCOL[q] * DH, (_QCOL[q] + 1) * DH)
        for m, pp in [("v", p["lsthm_v"]), ("a", p["lsthm_a"])]:
            w = np.zeros((XVA, DH), np.float32)
            if m == "v":
                w[0:DV] = pp["Wx"][:, qc]
            else:
                w[DV:DV + DA] = pp["Wx"][:, qc]
            w[XVA - 1] = pp["b"][qc]
            add(f"wxva_{m}_{q}", w)
        pl = p["lsthm_l"]
        add(f"wxl_0_{q}", pl["Wx"][0:128, qc])
        add(f"wxl_1_{q}", pl["Wx"][128:256, qc])
        wc = np.zeros((45, DH), np.float32)
        wc[0:44] = pl["Wx"][256:300, qc]
        wc[44] = pl["b"][qc]
        add(f"wxl_2_{q}", wc)
        for m, pp in [("v", p["lsthm_v"]), ("a", p["lsthm_a"]),
                      ("l", p["lsthm_l"])]:
            add(f"wh_{m}_{q}", pp["Wh"][:, qc])
            add(f"wz_{m}_{q}", pp["Wz"][:, qc])

    # cross modules
    for mi, cs in [(0, p["cs1"]), (1, p["cs2"])]:
        for k in range(K):
            add(f"wk_{mi}_{k}", cs["Wk"][k])
        add(f"wv2l_{mi}", cs["Wv2l"])          # [d2(hv feats), d1(out)]
        add(f"wl2v_{mi}", cs["Wl2v"])
        add(f"bl_{mi}", cs["bl"][None, :])     # [1, 128] K=1 bias tiles
        add(f"bv_{mi}", cs["bv"][None, :])

    add("ones32d", np.ones((128, 32), np.float32))
    wm = np.zeros((64, 1), np.float32)
    wm[0, 0] = 0.25
    wm[32, 0] = 0.25
    add("wmean64", wm)
    # broadcast rows [1, 128]
    add("wb_plain", np.full((1, 128), 1.0, np.float32))
    add("wb_07", np.full((1, 128), 0.7, np.float32))
    add("wb_03", np.full((1, 128), 0.3, np.float32))

    # heads
    for h, W1, W2 in [("z", p["zhat_W1"], p["zhat_W2"]),
                      ("g", p["gamma_W1"], p["gamma_W2"])]:
        for c in range(3):
            for o in range(2):
                add(f"w1_{h}_{c}_{o}",
                    W1[c * 128:(c + 1) * 128, o * 128:(o + 1) * 128])
        for c in range(2):
            add(f"w2_{h}_{c}", W2[c * 128:(c + 1) * 128, :])
    b2p_z = p["zhat_b1"] @ p["zhat_W2"] + p["zhat_b2"]
    b2p_g = p["gamma_b1"] @ p["gamma_W2"] + p["gamma_b2"]
    add("b2p_z", b2p_z[:, None])
    add("b2p_g", b2p_g[:, None])

    add("wp1", p["pred_W1"])
    add("bp1", p["pred_b1"][:, None])
    add("wp2", p["pred_W2"])
    add("bp2", p["pred_b2"][:, None])

    ncols = sum(a.shape[1] for _, a in tiles)
    w_np = np.zeros((128, ncols), np.float32)
    offsets = {}
    col = 0
    for name, arr in tiles:
        kk, mm = arr.shape
        w_np[0:kk, col:col + mm] = arr
        offsets[name] = (col, kk, mm)
        col += mm
    return w_np.astype(bf16), offsets


# ----------------------------------------------------------------- program --

_CACHE = {}


def build_program(T_steps=T, unroll=2):
    import concourse.bass as bass
    import concourse.bacc as bacc
    import concourse.tile as tile
    from concourse import mybir

    key = (T_steps, unroll)
    if key in _CACHE:
        return _CACHE[key]

    f32 = mybir.dt.float32
    b16 = mybir.dt.bfloat16
    AF = mybir.ActivationFunctionType
    ds = bass.ds

    # offsets only depend on shapes -> build with dummy params once
    dummy = _dummy_params()
    _, OFF = _pack_weights(dummy)
    NW = sum(v[2] for v in OFF.values())

    nc = bacc.Bacc()
    xl_d = nc.declare_dram_parameter("xl", [T_steps, XL_AUG, BS], b16,
                                     isOutput=False)
    xva_d = nc.declare_dram_parameter("xva", [T_steps, XVA, BS], b16,
                                      isOutput=False)
    w_d = nc.declare_dram_parameter("wts", [128, NW], b16, isOutput=False)
    out_d = nc.declare_dram_parameter("out", [3, BS], f32, isOutput=True)

    with tile.TileContext(nc) as tc:
        with (
            tc.tile_pool(name="wpool", bufs=1) as wpool,
            tc.tile_pool(name="state", bufs=1) as state,
            tc.tile_pool(name="xpool", bufs=3) as xpool,
            tc.tile_pool(name="sb", bufs=2) as sb,
            tc.tile_pool(name="pgates", bufs=1,
                         space=bass.MemorySpace.PSUM) as pgates,
            tc.tile_pool(name="pwork", bufs=5,
                         space=bass.MemorySpace.PSUM) as pwork,
        ):
            W = wpool.tile([128, NW], b16)
            nc.sync.dma_start(out=W, in_=w_d[:, :])

            def wt(name):
                c, kk, mm = OFF[name]
                return W[0:kk, c:c + mm]

            # persistent state
            c_all = state.tile([128, 3, BS], f32)    # c_v | c_a | c_l
            h_all = state.tile([128, 3, BS], b16)    # h_v | h_a | h_l
            z_sb = state.tile([128, BS], b16)
            ones_row = state.tile([1, BS], b16)
            nc.vector.memset(c_all, 0.0)
            nc.vector.memset(h_all, 0.0)
            nc.vector.memset(z_sb, 0.0)
            nc.vector.memset(ones_row, 1.0)
            h_v = h_all[:, 0, :]
            h_a = h_all[:, 1, :]
            h_l = h_all[:, 2, :]

            def step(t):
                # ---- input DMA
                xl_t = xpool.tile([128, 3, BS], b16, tag="xl")
                nc.sync.dma_start(
                    out=xl_t[:, 0:2, :],
                    in_=xl_d[ds(t, 1), 0:256, :].rearrange(
                        "o (c p) b -> (o p) c b", c=2, p=128))
                nc.sync.dma_start(
                    out=xl_t[0:45, 2, :],
                    in_=xl_d[ds(t, 1), 256:301, :].rearrange(
                        "o f b -> (o f) b"))
                xva_t = xpool.tile([XVA, BS], b16, tag="xva")
                nc.sync.dma_start(
                    out=xva_t,
                    in_=xva_d[ds(t, 1), :, :].rearrange("o f b -> (o f) b"))

                # ---- xv/xa input gating (sigmoid(Wx+b) * x)
                pva = pwork.tile([XVA, BS], f32, tag="work")
                nc.tensor.matmul(pva, wt("wg"), xva_t, start=True, stop=True)
                sva = sb.tile([XVA, BS], b16, tag="sva")
                nc.scalar.activation(sva, pva, AF.Sigmoid)
                xva_g = sb.tile([XVA, BS], b16, tag="xvag")
                nc.gpsimd.tensor_mul(xva_g, sva, xva_t)

                # ---- LSTHM gates in two 3-bank waves:
                # wave1 = [i_v i_a i_l f_v f_a f_l], wave2 = [o_v o_a o_l
                # g_v g_a g_l]
                def gate_mms(o, q, m):
                    if m in "va":
                        nc.tensor.matmul(o, wt(f"wxva_{m}_{q}"), xva_g,
                                         start=True, stop=False)
                    else:
                        nc.tensor.matmul(o, wt(f"wxl_0_{q}"), xl_t[:, 0, :],
                                         start=True, stop=False)
                        nc.tensor.matmul(o, wt(f"wxl_1_{q}"), xl_t[:, 1, :],
                                         start=False, stop=False)
                        nc.tensor.matmul(o, wt(f"wxl_2_{q}"),
                                         xl_t[0:45, 2, :],
                                         start=False, stop=False)
                    nc.tensor.matmul(o, wt(f"wh_{m}_{q}"),
                                     h_all[:, "val".index(m), :],
                                     start=False, stop=False)
                    nc.tensor.matmul(o, wt(f"wz_{m}_{q}"), z_sb,
                                     start=False, stop=True)

                G1 = pgates.tile([128, 6, BS], f32, tag="gates")
                for si, (q, m) in enumerate(
                        [(q, m) for q in ["i", "f"] for m in "val"]):
                    gate_mms(G1[:, si, :], q, m)
                sig_if = sb.tile([128, 6, BS], b16, tag="sigif")
                nc.scalar.activation(sig_if, G1, AF.Sigmoid)

                G2 = pgates.tile([128, 6, BS], f32, tag="gates")
                for si, (q, m) in enumerate(
                        [(q, m) for q in ["o", "g"] for m in "val"]):
                    gate_mms(G2[:, si, :], q, m)
                so_t = sb.tile([128, 3, BS], b16, tag="sot")
                nc.scalar.activation(so_t, G2[:, 0:3, :], AF.Sigmoid)
                tg = sb.tile([128, 3, BS], b16, tag="tg")
                nc.scalar.activation(tg, G2[:, 3:6, :], AF.Tanh)

                d1 = sb.tile([128, 3, BS], f32, tag="d1")
                nc.vector.tensor_mul(d1, sig_if[:, 3:6, :], c_all)  # f*c
                d2 = sb.tile([128, 3, BS], f32, tag="d2")
                nc.vector.tensor_mul(d2, sig_if[:, 0:3, :], tg)     # i*tanh(g)
                nc.vector.tensor_add(c_all, d1, d2)                 # c_new
                tc_t = sb.tile([128, 3, BS], b16, tag="tc")
                nc.scalar.activation(tc_t, c_all, AF.Tanh)
                nc.vector.tensor_mul(h_all, so_t, tc_t)             # h = o*..

                # ---- cross modules: bilinear gate dots
                # dots_m [33, 2, BS]: partition 0 holds (k0|k1), partition 32
                # holds (k2|k3) (matmul out base partition must be 32-aligned)
                Gm_all = pwork.tile([1, 2, BS], f32, tag="work")
                for mi, partner in [(0, h_v), (1, h_a)]:
                    prod = sb.tile([128, 4, BS], b16, tag=f"prod{mi}")
                    for half in range(2):
                        tmp = pwork.tile([128, 2, BS], f32, tag="work")
                        for j in range(2):
                            kk = half * 2 + j
                            nc.tensor.matmul(tmp[:, j, :], wt(f"wk_{mi}_{kk}"),
                                             h_l, start=True, stop=True)
                        nc.vector.tensor_mul(prod[:, half * 2:half * 2 + 2, :],
                                             tmp, partner_rep(partner))
                        # dots_h [64, BS]: k(2*half) at rows 0-31, k(2*half+1)
                        # at rows 32-63 (all-ones lhsT -> 32 equal rows)
                        dots = pwork.tile([64, BS], f32, tag="work")
                        for j in range(2):
                            nc.tensor.matmul(
                                dots[j * 32:j * 32 + 32, :],
                                wt("ones32d"),
                                prod[:, half * 2 + j, :],
                                start=True, stop=True)
                        td = sb.tile([64, BS], b16, tag=f"td{mi}{half}")
                        nc.scalar.activation(td, dots, AF.Tanh,
                                             scale=float(1.0 / np.sqrt(DH)))
                        # 0.25*(row0 + row32), accumulated over both halves
                        nc.tensor.matmul(
                            Gm_all[:, mi, :], wt("wmean64"), td,
                            start=(half == 0), stop=(half == 1))
                g_sb = sb.tile([1, 2, BS], b16, tag="g")
                nc.scalar.activation(g_sb, Gm_all, AF.Sigmoid)

                # broadcasts: bc0 = [g1 | g2], bc1 = [0.7g1 | 0.3g2]
                bc0 = pwork.tile([128, 2, BS], f32, tag="work")
                nc.tensor.matmul(bc0.rearrange("p k b -> p (k b)"),
                                 wt("wb_plain"),
                                 g_sb.rearrange("p k b -> p (k b)"),
                                 start=True, stop=True)
                bc1 = pwork.tile([128, 2, BS], f32, tag="work")
                nc.tensor.matmul(bc1[:, 0, :], wt("wb_07"), g_sb[:, 0, :],
                                 start=True, stop=True)
                nc.tensor.matmul(bc1[:, 1, :], wt("wb_03"), g_sb[:, 1, :],
                                 start=True, stop=True)

                # cross tanh terms: ct0 = [t1v | tv], ct1 = [t2a | ta]
                cts = []
                for mi, partner in [(0, h_v), (1, h_a)]:
                    ct = pwork.tile([128, 2, BS], f32, tag="work")
                    nc.tensor.matmul(ct[:, 0, :], wt(f"wv2l_{mi}"), partner,
                                     start=True, stop=False)
                    nc.tensor.matmul(ct[:, 0, :], wt(f"bl_{mi}"), ones_row,
                                     start=False, stop=True)
                    nc.tensor.matmul(ct[:, 1, :], wt(f"wl2v_{mi}"), h_l,
                                     start=True, stop=False)
                    nc.tensor.matmul(ct[:, 1, :], wt(f"bv_{mi}"), ones_row,
                                     start=False, stop=True)
                    cth = sb.tile([128, 2, BS], b16, tag=f"ct{mi}")
                    nc.scalar.activation(cth, ct, AF.Tanh)
                    cts.append(cth)

                # combines
                p1 = sb.tile([128, BS], b16, tag="p1")
                nc.vector.tensor_mul(p1, bc1[:, 0, :], cts[0][:, 0, :])
                p2 = sb.tile([128, BS], b16, tag="p2")
                nc.vector.tensor_mul(p2, bc1[:, 1, :], cts[1][:, 0, :])
                s12 = sb.tile([128, BS], b16, tag="s12")
                nc.gpsimd.tensor_add(s12, p1, p2)
                lh = sb.tile([128, BS], b16, tag="lh")
                nc.gpsimd.tensor_add(lh, h_l, s12)
                q1 = sb.tile([128, BS], b16, tag="q1")
                nc.vector.tensor_mul(q1, bc0[:, 0, :], cts[0][:, 1, :])
                vh = sb.tile([128, BS], b16, tag="vh")
                nc.gpsimd.tensor_add(vh, h_v, q1)
                q2 = sb.tile([128, BS], b16, tag="q2")
                nc.vector.tensor_mul(q2, bc0[:, 1, :], cts[1][:, 1, :])
                ah = sb.tile([128, BS], b16, tag="ah")
                nc.gpsimd.tensor_add(ah, h_a, q2)

                # heads: layer1 (no nonlinearity; b1 folded into layer2 bias)
                chunks = [lh, vh, ah]
                h1ps = []
                for h in "zg":
                    h1p = pwork.tile([128, 2, BS], f32, tag="work")
                    for o in range(2):
                        for c in range(3):
                            nc.tensor.matmul(h1p[:, o, :], wt(f"w1_{h}_{c}_{o}"),
                                             chunks[c], start=(c == 0),
                                             stop=(c == 2))
                    h1ps.append(h1p)
                h1s = sb.tile([128, 4, BS], b16, tag="h1s")
                nc.scalar.activation(h1s[:, 0:2, :], h1ps[0], AF.Copy)
                nc.vector.tensor_copy(h1s[:, 2:4, :], h1ps[1])
                h2p = pwork.tile([128, 2, BS], f32, tag="work")
                for hi, h in enumerate("zg"):
                    for c in range(2):
                        nc.tensor.matmul(h2p[:, hi, :], wt(f"w2_{h}_{c}"),
                                         h1s[:, hi * 2 + c, :],
                                         start=(c == 0), stop=(c == 1))
                zhat = sb.tile([128, BS], b16, tag="zhat")
                nc.scalar.activation(zhat, h2p[:, 0, :], AF.Tanh,
                                     bias=wt("b2p_z"))
                gam = sb.tile([128, BS], b16, tag="gam")
                nc.scalar.activation(gam, h2p[:, 1, :], AF.Sigmoid,
                                     bias=wt("b2p_g"))
                nc.vector.tensor_mul(z_sb, gam, zhat)

            def partner_rep(h):
                # view [128, BS] h slice as [128, 2, BS], middle dim stride 0
                return bass.AP(tensor=h.tensor, offset=h.offset,
                               ap=[h.ap[0], [0, 2], h.ap[1]])

            if unroll == 0:
                # static full unroll: no dynamic loop machinery at all
                for t in range(T_steps):
                    step(t)
            elif unroll <= 1:
                with tc.For_i(0, T_steps, 1, staggered_reset=True) as t:
                    step(t)
            else:
                assert T_steps % unroll == 0
                with tc.For_i(0, T_steps, unroll,
                              staggered_reset=True) as t0:
                    for u in range(unroll):
                        step(t0 + u)

            # prediction tail
            pp = pwork.tile([128, BS], f32, tag="work")
            nc.tensor.matmul(pp, wt("wp1"), z_sb, start=True, stop=True)
            y1 = sb.tile([128, BS], b16, tag="y1")
            nc.scalar.activation(y1, pp, AF.Relu,
                                 bias=wt("bp1"))
            po = pwork.tile([3, BS], f32, tag="work")
            nc.tensor.matmul(po, wt("wp2"), y1, start=True, stop=True)
            oo = sb.tile([3, BS], f32, tag="oo")
            nc.scalar.activation(oo, po, AF.Identity,
                                 bias=wt("bp2"))
            nc.sync.dma_start(out=out_d[:, :], in_=oo)

    nc.finalize()
    _CACHE[key] = nc
    return nc


def _dummy_params():
    def lsthm_p(din):
        return {"Wx": np.zeros((din, 4 * DH), np.float32),
                "Wh": np.zeros((DH, 4 * DH), np.float32),
                "Wz": np.zeros((ZD, 4 * DH), np.float32),
                "b": np.zeros((4 * DH,), np.float32)}

    def cross_p():
        return {"Wk": np.zeros((K, DH, DH), np.float32),
                "Wv2l": np.zeros((DH, DH), np.float32),
                "bl": np.zeros((DH,), np.float32),
                "Wl2v": np.zeros((DH, DH), np.float32),
                "bv": np.zeros((DH,), np.float32)}

    return {
        "gate_v_W": np.zeros((DV, DV), np.float32),
        "gate_v_b": np.zeros((DV,), np.float32),
        "gate_a_W": np.zeros((DA, DA), np.float32),
        "gate_a_b": np.zeros((DA,), np.float32),
        "lsthm_l": lsthm_p(DL), "lsthm_a": lsthm_p(DA), "lsthm_v": lsthm_p(DV),
        "cs1": cross_p(), "cs2": cross_p(),
        "zhat_W1": np.zeros((3 * DH, ZF1), np.float32),
        "zhat_b1": np.zeros((ZF1,), np.float32),
        "zhat_W2": np.zeros((ZF1, ZD), np.float32),
        "zhat_b2": np.zeros((ZD,), np.float32),
        "gamma_W1": np.zeros((3 * DH, ZF2), np.float32),
        "gamma_b1": np.zeros((ZF2,), np.float32),
        "gamma_W2": np.zeros((ZF2, ZD), np.float32),
        "gamma_b2": np.zeros((ZD,), np.float32),
        "pred_W1": np.zeros((ZD, FD), np.float32),
        "pred_b1": np.zeros((FD,), np.float32),
        "pred_W2": np.zeros((FD, 3), np.float32),
        "pred_b2": np.zeros((3,), np.float32),
    }


# ------------------------------------------------------------------ driver --

def prep_inputs(x_l, x_v, x_a, T_steps=T):
    """Build per-core host tensors: xl [T, 301, BS] bf16, xva [T, 122, BS]."""
    x_l = np.asarray(x_l, np.float32)
    x_v = np.asarray(x_v, np.float32)
    x_a = np.asarray(x_a, np.float32)
    xls, xvas = [], []
    for ci in range(NCORES):
        sl = slice(ci * BS, (ci + 1) * BS)
        # [BS, T, D] -> [T, D, BS]
        xl = np.ascontiguousarray(
            x_l[sl, :T_steps].transpose(1, 2, 0))
        xl_aug = np.empty((T_steps, XL_AUG, BS), np.float32)
        xl_aug[:, :DL] = xl
        xl_aug[:, DL] = 1.0
        xv = x_v[sl, :T_steps].transpose(1, 2, 0)
        xa = x_a[sl, :T_steps].transpose(1, 2, 0)
        xva = np.empty((T_steps, XVA, BS), np.float32)
        xva[:, 0:DV] = xv
        xva[:, DV:DV + DA] = xa
        xva[:, XVA - 1] = 1.0
        xls.append(xl_aug.astype(bf16))
        xvas.append(xva.astype(bf16))
    return xls, xvas


LAST_EXEC_NS = None
LAST_PROFILE = None
UNROLL = int(os.environ.get("KERNEL_UNROLL", "2"))


def bench(x_l, x_v, x_a, params, iters=20):
    """Time warm executions with device-resident inputs.

    Returns (avg_ns, min_ns) per kernel execution across the 8 cores.
    """
    import time
    import jax
    from jax.sharding import Mesh, PartitionSpec, NamedSharding
    from jax.experimental.shard_map import shard_map
    from concourse import bass2jax, mybir
    from concourse.bass2jax import _bass_exec_p, install_neuronx_cc_hook, \
        partition_id_tensor

    install_neuronx_cc_hook()
    nc = build_program(T, unroll=UNROLL)
    w_np, _ = _pack_weights(params)
    xls, xvas = prep_inputs(x_l, x_v, x_a)
    in_maps = [{"xl": xls[ci], "xva": xvas[ci], "wts": w_np}
               for ci in range(NCORES)]

    partition_name = (nc.partition_id_tensor.name
                      if nc.partition_id_tensor else None)
    in_names, out_names, out_avals, zero_outs = [], [], [], []
    for alloc in nc.m.functions[0].allocations:
        if not isinstance(alloc, mybir.MemoryLocationSet):
            continue
        name = alloc.memorylocations[0].name
        if alloc.kind == "ExternalInput":
            if name != partition_name:
                in_names.append(name)
        elif alloc.kind == "ExternalOutput":
            out_names.append(name)
            shape = tuple(alloc.tensor_shape)
            dtype = mybir.dt.np(alloc.dtype)
            out_avals.append(jax.core.ShapedArray(shape, dtype))
            zero_outs.append(np.zeros(shape, dtype))
    n_params = len(in_names)
    n_outs = len(out_avals)
    in_names_all = list(in_names) + out_names
    if partition_name is not None:
        in_names_all.append(partition_name)

    def _body(*args):
        operands = list(args)
        if partition_name is not None:
            operands.append(partition_id_tensor())
        outs = _bass_exec_p.bind(
            *operands, out_avals=tuple(out_avals),
            in_names=tuple(in_names_all), out_names=tuple(out_names),
            lowering_input_output_aliases=(), sim_require_finite=True,
            sim_require_nnan=True, nc=nc)
        return tuple(outs)

    devices = jax.devices()[:NCORES]
    mesh = Mesh(np.asarray(devices), ("core",))
    in_specs = (PartitionSpec("core"),) * (n_params + n_outs)
    out_specs = (PartitionSpec("core"),) * len(out_names)
    sharded = jax.jit(
        shard_map(_body, mesh=mesh, in_specs=in_specs, out_specs=out_specs,
                  check_rep=False),
        donate_argnums=tuple(range(n_params, n_params + n_outs)),
        keep_unused=True)
    shard = NamedSharding(mesh, PartitionSpec("core"))
    concat_in = [
        jax.device_put(
            np.concatenate([np.asarray(in_maps[c][nm])
                            for c in range(NCORES)], axis=0), shard)
        for nm in in_names]

    def once():
        zeros = [np.zeros((NCORES * z.shape[0], *z.shape[1:]), z.dtype)
                 for z in zero_outs]
        out = sharded(*concat_in, *zeros)
        jax.block_until_ready(out)
        return out

    once()  # warmup / compile
    times = []
    for _ in range(iters):
        t0 = time.perf_counter()
        once()
        times.append((time.perf_counter() - t0) * 1e9)
    return float(np.mean(times)), float(np.min(times))


def kernel(x_l, x_v, x_a, IsLastBatch=None, params=None):
    global LAST_EXEC_NS, LAST_PROFILE
    from concourse.bass_utils import run_bass_kernel_spmd

    trace = os.environ.get("KERNEL_TRACE", "0") == "1"
    nc = build_program(T, unroll=UNROLL)
    w_np, _ = _pack_weights(params)
    xls, xvas = prep_inputs(x_l, x_v, x_a)
    in_maps = [{"xl": xls[ci], "xva": xvas[ci], "wts": w_np}
               for ci in range(NCORES)]
    res = run_bass_kernel_spmd(nc, in_maps, list(range(NCORES)), trace=trace)
    LAST_EXEC_NS = getattr(res, "exec_time_ns", None)
    LAST_PROFILE = getattr(res, "profile_json", None)
    outs = [np.asarray(res.results[ci]["out"]).T for ci in range(NCORES)]
    return np.concatenate(outs, axis=0).astype(np.float32)
